# revision 24
# baseline (speedup 1.0000x reference)
"""Trainium2 Bass kernel for nn_FCNet (dense MLP, 8 layers).

Network: x[B,P,64] -> Linear(64->1024) -> 7x (ReLU -> Linear(1024->1024))
with B=4, P=2048 (8192 rows total), fp32 reference.

Strategy (8 NeuronCores, data-parallel):
  - Shard the 8192 rows across 8 cores (1024 rows each); replicate weights.
  - Feature-major on-chip layout: h[t] lives as [128 partitions, 8 feature
    chunks, rows] so every layer is PSUM-accumulated matmuls with lhsT =
    W.T tiles (pre-transposed on host) and rhs = h — no on-chip transposes.
  - Matmuls run in fp16 (1 cycle/row on the PE vs 4 for fp32) with fp32
    PSUM accumulation; per-layer rel-RMS error ~3e-4, ~8e-4 end to end.
  - All 7 hidden-layer weights (fp16) stay resident in SBUF (112 KB per
    partition), DMA'd once and prefetched under layer-0 compute. Initial
    DMAs are spread across the SP/ACT/Pool queues so descriptor
    generation and the single-slot HWDGE don't serialize the prologue.
  - Rows are processed in two blocks of 512 (PSUM bank = 512 fp32), and the
    two blocks are interleaved within each layer so the PE never waits for
    the previous layer's last evacuation.
  - PSUM -> SBUF evacuation is a fused bias+ReLU tensor_scalar on the
    Vector engine (out = max(psum + b, 0), written as fp16) — measured to
    keep up with the PE at zero cost; the last layer evacuates via the
    otherwise-idle Scalar engine (Identity+bias) into a persistent SBUF
    staging tile as fp16, which (a) frees the PSUM bank immediately —
    decoupling the next iteration's layer-0 matmuls from output-DMA
    descriptor pacing — and (b) halves output DMA bytes. Output DMAs
    alternate between the SP queue (HWDGE) and the Pool queue (SWDGE,
    bypasses HWDGE) so no single descriptor path paces the epilogue.

TimelineSim (2.4 GHz): steady-state body = ~194.4 us/iter, matching the
912-matmul fp16 streaming floor of 194.3 us (the layer-0/layer-1
interleave in emit_body removed the last ~2.8 us of body-boundary PSUM
stretch, confirmed -2.75 us/iter on HW); single-shot ~209 us (HAM
cold-clock ramp + prologue DMA + epilogue drain). On HW the shared chip
flips between a ~2.4 GHz state (~222 ns/MM -> ~203 us/iter) and a
~2.0 GHz state (~265 ns/MM -> ~243 us/iter) on minute scales
(co-tenant/thermal); a pure 912-matmul stream with no evacs/DMAs,
interleaved burst-for-burst with this kernel, measures within ~1.2% of
it in either state — i.e. the kernel runs AT the PE streaming floor of
whatever clock state the chip is in, and everything else is hidden.
Relative RMS error vs the fp32 reference: ~9e-4.
"""

import numpy as np

import concourse.bacc as bacc
import concourse.mybir as mybir
import concourse.tile as tile
from concourse.bass_utils import run_bass_kernel_spmd

P = 128          # partitions
VEC = 1024       # hidden/output width
IN = 64          # input feature dim
K_LAYERS = 7     # hidden Linear layers after the first
N_CORES = 8
ROWS_PER_CORE = 1024
RB = 512         # row block (one PSUM bank of fp32)
NRB = ROWS_PER_CORE // RB
NC_FEAT = VEC // P  # 8 feature chunks

_cached = {}


def _evac_relu(nc, out_ap, ps, bias_ap):
    """PSUM -> SBUF fp16, out = relu(psum + bias), on the Vector engine.

    One tensor_scalar op: (psum + bias) then max(., 0). Measured: DVE keeps
    up with the PE at zero cost, while ACT-engine activation adds ~12 us of
    pipeline stall per pass (higher instruction latency blocks PSUM reuse).
    """
    nc.vector.tensor_scalar(
        out_ap, ps[:], bias_ap, 0.0,
        mybir.AluOpType.add, mybir.AluOpType.max)


def _evac_final(nc, o_ap, ps, bias_ap):
    """Final layer PSUM -> SBUF staging fp16, out = psum + bias (no relu),
    on the otherwise-idle Scalar engine."""
    nc.scalar.activation(
        o_ap, ps[:], mybir.ActivationFunctionType.Identity, bias=bias_ap)


def _build_nc(loop=False, n_body=1):
    """Per-core program: out[1024,1024] (feature-major, fp16) = MLP(x shard).

    loop=True adds a `niter` input and wraps n_body copies of the layer
    stack in a runtime For_i — used only for hardware timing (the
    per-iteration slope isolates on-device time from host/dispatch
    overhead; n_body>1 amortizes the loop's all-engine barrier). The
    total MLP iterations executed = niter * n_body.
    """
    nc = bacc.Bacc("TRN2", target_bir_lowering=False, debug=False)
    f16, f32 = mybir.dt.float16, mybir.dt.float32

    xT = nc.dram_tensor("xT", [P, ROWS_PER_CORE], f16, kind="ExternalInput")
    w0T = nc.dram_tensor("w0T", [P, VEC], f16, kind="ExternalInput")
    whT = nc.dram_tensor("whT", [K_LAYERS, VEC, VEC], f16, kind="ExternalInput")
    bias = nc.dram_tensor("bias", [P, (K_LAYERS + 1) * NC_FEAT], f32,
                          kind="ExternalInput")
    if loop:
        niter = nc.dram_tensor("niter", [1, 1], mybir.dt.uint32,
                               kind="ExternalInput")
    out = nc.dram_tensor("out", [VEC, ROWS_PER_CORE], f16, kind="ExternalOutput")
    out3 = out.rearrange("(oc p) r -> p oc r", p=P)

    with tile.TileContext(nc) as tc:
        with (
            tc.tile_pool(name="wpool", bufs=1) as wpool,
            tc.tile_pool(name="hpool", bufs=1) as hpool,
            tc.tile_pool(name="psum", bufs=8, space="PSUM") as psum,
        ):
            x_sb = wpool.tile([P, ROWS_PER_CORE], f16, tag="x")
            w0_sb = wpool.tile([P, VEC], f16, tag="w0")
            b_sb = wpool.tile([P, (K_LAYERS + 1) * NC_FEAT], f32, tag="b")
            wh_sb = wpool.tile([P, K_LAYERS * NC_FEAT, VEC], f16, tag="wh")

            # prologue loads: spread across engine DMA queues so the
            # first matmul (needs x + w0) isn't queued behind 56 weight
            # chunks on one sequencer / the single-slot HWDGE.
            nc.sync.dma_start(x_sb[:], xT[:])
            nc.scalar.dma_start(w0_sb[:], w0T[:])
            nc.gpsimd.dma_start(b_sb[:], bias[:])
            dma_engines = (nc.sync, nc.scalar, nc.gpsimd)
            for l in range(K_LAYERS):
                for kc in range(NC_FEAT):
                    eng = dma_engines[(l * NC_FEAT + kc) % 3]
                    eng.dma_start(
                        wh_sb[:, l * NC_FEAT + kc, :],
                        whT[l, kc * P:(kc + 1) * P, :],
                    )

            # ping-pong activation buffers, one pair per row block
            h = [[hpool.tile([P, NC_FEAT, RB], f16, tag=f"h_{rb}_{s}",
                             name=f"h_{rb}_{s}")
                  for s in range(2)] for rb in range(NRB)]
            # persistent fp16 staging for the final layer's outputs: the
            # ACT evac frees the PSUM bank immediately; the output DMA
            # reads from here at its own pace.
            o_st = [hpool.tile([P, NC_FEAT, RB], f16, tag=f"o_{rb}",
                               name=f"o_{rb}") for rb in range(NRB)]

            def emit_body():
                # Layer-0 contracts only IN(=64, zero-padded to 128) real
                # features: one matmul per (rb, oc) group.
                def l0_group(rb, oc):
                    ps = psum.tile([P, RB], f32, tag="ps", name="ps0")
                    nc.tensor.matmul(
                        ps[:], w0_sb[:, oc * P:(oc + 1) * P],
                        x_sb[:, rb * RB:(rb + 1) * RB],
                        start=True, stop=True)
                    _evac_relu(nc, h[rb][0][:, oc, :], ps,
                               b_sb[:, oc:oc + 1])

                def hid_group(j, rb, oc):
                    wbase = (j - 1) * NC_FEAT
                    bcol = j * NC_FEAT
                    src, dst = (j + 1) % 2, j % 2
                    h_in = h[rb][src]
                    ps = psum.tile([P, RB], f32, tag="ps", name="ps")
                    for kc in range(NC_FEAT):
                        nc.tensor.matmul(
                            ps[:], wh_sb[:, wbase + kc, oc * P:(oc + 1) * P],
                            h_in[:, kc, :],
                            start=(kc == 0), stop=(kc == NC_FEAT - 1))
                    if j < K_LAYERS:
                        _evac_relu(nc, h[rb][dst][:, oc, :], ps,
                                   b_sb[:, bcol + oc:bcol + oc + 1])
                    else:
                        _evac_final(nc, o_st[rb][:, oc, :], ps,
                                    b_sb[:, bcol + oc:bcol + oc + 1])

                # Layer-0's rb1 groups are interleaved into layer-1's rb0
                # group stream: a body then opens with only 8 (not 16)
                # rapid single-matmul PSUM grabs, so at the body boundary
                # the previous body's final-layer bank releases keep pace
                # and the layer-0 matmuls never stretch. Sim + HW both
                # measure this at the 912-matmul PE streaming floor
                # (-2.8 us/iter vs the flat order).
                for oc in range(NC_FEAT):
                    l0_group(0, oc)
                for oc in range(NC_FEAT):
                    hid_group(1, 0, oc)
                    l0_group(1, oc)
                for oc in range(NC_FEAT):
                    hid_group(1, 1, oc)
                for j in range(2, K_LAYERS + 1):
                    for rb in range(NRB):
                        for oc in range(NC_FEAT):
                            hid_group(j, rb, oc)
                        if j == K_LAYERS:
                            eng = nc.sync if rb == 0 else nc.gpsimd
                            eng.dma_start(
                                out3[:, :, rb * RB:(rb + 1) * RB],
                                o_st[rb][:])

            if loop:
                n_sb = wpool.tile([1, 1], mybir.dt.uint32, tag="niter")
                nc.sync.dma_start(n_sb[:], niter[:])
                n_rv = nc.values_load(n_sb[0:1, 0:1], max_val=1 << 20,
                                      skip_runtime_bounds_check=True)
                with tc.For_i(0, n_rv, 1):
                    for _ in range(n_body):
                        emit_body()
            else:
                for _ in range(n_body):
                    emit_body()
    nc.compile()
    return nc


def _get_nc(loop=False, n_body=1):
    key = ("nc", loop, n_body)
    if key not in _cached:
        _cached[key] = _build_nc(loop=loop, n_body=n_body)
    return _cached[key]


def build_in_maps(x, W0, b0, Wh, bh):
    x = np.asarray(x, dtype=np.float32)
    W0 = np.asarray(W0, dtype=np.float32)
    b0 = np.asarray(b0, dtype=np.float32)
    Wh = np.asarray(Wh, dtype=np.float32)
    bh = np.asarray(bh, dtype=np.float32)
    B, Pp, _ = x.shape
    rows = B * Pp
    per = rows // N_CORES

    xf = x.reshape(rows, IN)
    w0T = np.zeros((P, VEC), dtype=np.float16)
    w0T[:IN] = W0.T.astype(np.float16)
    whT = np.ascontiguousarray(Wh.transpose(0, 2, 1)).astype(np.float16)

    bias = np.zeros((P, (K_LAYERS + 1) * NC_FEAT), dtype=np.float32)
    bias[:, :NC_FEAT] = b0.reshape(NC_FEAT, P).T
    for l in range(K_LAYERS):
        bias[:, (l + 1) * NC_FEAT:(l + 2) * NC_FEAT] = bh[l].reshape(NC_FEAT, P).T

    in_maps = []
    for c in range(N_CORES):
        xT = np.zeros((P, per), dtype=np.float16)
        xT[:IN] = xf[c * per:(c + 1) * per].T.astype(np.float16)
        in_maps.append({"xT": xT, "w0T": w0T, "whT": whT, "bias": bias})
    return in_maps


def kernel(x, W0, b0, Wh, bh):
    B, Pp, _ = np.asarray(x).shape
    in_maps = build_in_maps(x, W0, b0, Wh, bh)
    res = run_bass_kernel_spmd(_get_nc(), in_maps, list(range(N_CORES)))
    outs = [res.results[c]["out"].T for c in range(N_CORES)]  # [rows, VEC]
    return np.concatenate(outs, axis=0).reshape(B, Pp, VEC).astype(np.float32)
